# revision 29
# baseline (speedup 1.0000x reference)
"""Trainium2 Bass kernel for causal multi-head attention.

Problem: B=4, S=2048, D=1024, H=16 (head_dim 64), fp32.
  qkv = x @ w_attn + b_attn ; causal SDPA ; out @ w_proj + b_proj

Sharding (8 cores): data-parallel over B (4) x tensor-parallel over head
halves (2). Core c handles batch b=c//2, heads [8*(c%2), 8*(c%2)+8).
Each core computes its qkv slice, its heads' attention, and a partial
output projection (its heads' rows of w_proj); the host sums the two
partials per batch. b_proj is added on even cores (odd cores get zeros).

Device dataflow (per core) — fully software-pipelined. The qkv
projection (P1), attention (P2), and output projection (P3) are emitted
interleaved per query block so the Tile scheduler can fill TensorE with
P1/P3 GEMM work while P2 is paced by the exp() on ScalarE (ACT runs at
1.2 GHz and the softmax exp is ~160us of it — longer than P2's own PE
work). Schedule per query block g (512 queries):
  P2(g) over its 4(g+1) key tiles + P1 chunks 2g+2,2g+3 (qkv for the
  NEXT block's queries/keys) + P3 groups of block g-1 as PE filler.

Layouts: qT,kT = (w_q|k)^T @ x^T -> [e, s] (e on partitions), bf16;
v stored augmented as [ones | v+bias] bf16 so a single M=128 matmul
yields attn@v and the softmax denominator (fp32r PSUM row trick);
scores fp32 in PSUM -> exp on ACT -> eAB bf16; block-causal via
narrowed moving ranges + triangular multiply on diagonal tiles;
normalization (reciprocal * numerator) on DVE -> aoT bf16;
P3 y = aoT.T @ wp (bf16) + bp, DMA out.

P1 matmuls run float32r (E8M11, full rate at moving>=256, ~1e-7 err);
attention and P3 run bf16 (inputs O(1), rel err ~1e-3 << 2e-2 budget).
"""

import math
import os

import numpy as np
import ml_dtypes

import concourse.bass as bass
import concourse.mybir as mybir
import concourse.tile as tile
from concourse import bacc

last_exec_time_ns = None

B, S, D, H = 4, 2048, 1024, 16
HD = D // H          # 64
HPC = H // 2         # heads per core = 8
EC = HPC * HD        # per-core qkv slice width = 512
NP = 4               # head pairs per core
QB = 512             # query block width
KT = 128             # key tile
N_QB = S // QB       # 4
N_KT = S // KT       # 16
DT = D // 128        # 8 contraction tiles
CB = 512             # P1 s-chunk width (512 so matmul N > LDWEIGHTS time)

F32 = mybir.dt.float32
F32R = mybir.dt.float32r
BF16 = mybir.dt.bfloat16

_nc_cache: dict = {}


def _round_fp32r(x: np.ndarray) -> np.ndarray:
    """Round-to-nearest-even fp32 -> fp32r (E8M11: low 12 mantissa bits 0)."""
    u = np.ascontiguousarray(x, dtype=np.float32).view(np.uint32)
    u2 = (u + 0x7FF + ((u >> 12) & 1)) & np.uint32(0xFFFFF000)
    return u2.view(np.float32)


def _build(causal: bool):
    nc = bacc.Bacc("TRN2", target_bir_lowering=False)
    xT = nc.dram_tensor("xT", [D, S], F32R, kind="ExternalInput")
    wqkv = nc.dram_tensor("wqkv", [D, 3 * EC], F32R, kind="ExternalInput")
    bqk = nc.dram_tensor("bqk", [128, 2 * EC // 128], F32, kind="ExternalInput")
    bv = nc.dram_tensor("bv", [1, EC], F32, kind="ExternalInput")
    wp = nc.dram_tensor("wp", [EC, D], BF16, kind="ExternalInput")
    bp = nc.dram_tensor("bp", [1, D], F32, kind="ExternalInput")
    tri = nc.dram_tensor("tri", [128, 128], BF16, kind="ExternalInput")
    y = nc.dram_tensor("y", [S, D], F32, kind="ExternalOutput")

    n_qk_et = 2 * EC // 128   # 8 e-tiles for q+k
    scale = 1.0 / math.sqrt(HD)
    N_CH = S // CB            # 8 P1 chunks

    with tile.TileContext(nc) as tc:
        with (
            tc.tile_pool(name="persist", bufs=1) as persist,
            tc.tile_pool(name="qkvp", bufs=1) as qkvp,
            tc.tile_pool(name="p1x", bufs=2) as p1x,
            tc.tile_pool(name="p2e", bufs=8) as p2e,
            tc.tile_pool(name="p2r", bufs=1) as p2r,
            tc.tile_pool(name="p3y", bufs=2) as p3y,
            tc.tile_pool(name="psS", bufs=2, space="PSUM") as psS,
            tc.tile_pool(name="psO", bufs=1, space="PSUM") as psO,
            tc.tile_pool(name="psP", bufs=2, space="PSUM") as psP,
        ):
            # ---- persistent SBUF ----
            bqk_sb = persist.tile([128, n_qk_et], F32, tag="bqk_sb")
            tri_sb = persist.tile([128, 128], BF16, tag="tri_sb")
            bv_sb = persist.tile([128, EC], F32, tag="bv_sb")
            bp_sb = persist.tile([128, D], F32, tag="bp_sb")
            w_sb = persist.tile([128, DT, 3 * EC], F32R, tag="w_sb")
            wp_sb = persist.tile([128, EC // 128, D], BF16, tag="wp_sb")

            qT = qkvp.tile([128, NP, S], BF16, tag="qT")
            kT = qkvp.tile([128, NP, S], BF16, tag="kT")
            # augmented v: per head h and key tile t, [ones | v_h+bias]
            va = qkvp.tile([128, N_KT, HPC, 128], BF16, tag="va")
            aoT = qkvp.tile([128, NP, S], BF16, tag="aoT")
            xts0 = p1x.tile([128, DT, CB], F32R, tag="xts")

            # ---- startup DMA schedule. The weights load et-major (one
            # strided 0.5MB DMA per 128-col block, all 8 dt rows at once)
            # in the order P1 consumes them [q0, k0, v, ..., q3, k3], so
            # the first matmul group starts after ~1MB instead of 6.3MB.
            # x chunk 0 rides the gpsimd queue in parallel. Small consts
            # that early evacs need (bqk, tri) go first on scalar.
            nc.scalar.dma_start(out=bqk_sb, in_=bqk.ap())
            nc.scalar.dma_start(out=tri_sb, in_=tri.ap())
            wr = wqkv.ap().rearrange("(t p) c -> p t c", p=128)

            def w_block(eng, cb):
                eng.dma_start(
                    out=w_sb[:, :, cb * 128:(cb + 1) * 128],
                    in_=wr[:, :, cb * 128:(cb + 1) * 128],
                )

            def x0_slab(eng, dt):
                eng.dma_start(
                    out=xts0[:, dt, :],
                    in_=xT.ap()[dt * 128:(dt + 1) * 128, 0:CB],
                )

            # critical set first, striped over the sync/gpsimd queues —
            # the scalar queue carries ONLY bqk/tri so the exp stream is
            # never queue-blocked behind DMA issues: q0/k0 weight blocks
            # + the 8 x slabs of chunk 0, then q1/k1, v, the rest.
            w_block(nc.sync, 0)           # q0
            x0_slab(nc.gpsimd, 0)
            x0_slab(nc.sync, 1)
            w_block(nc.gpsimd, 4)         # k0
            x0_slab(nc.gpsimd, 2)
            x0_slab(nc.sync, 3)
            x0_slab(nc.gpsimd, 5)
            x0_slab(nc.sync, 4)
            x0_slab(nc.gpsimd, 6)
            x0_slab(nc.sync, 7)
            for i, cb in enumerate([1, 5, 8, 9, 10, 11, 2, 6, 3, 7]):
                w_block(nc.sync if i % 2 == 0 else nc.gpsimd, cb)
            nc.sync.dma_start(out=bv_sb, in_=bv.ap().to_broadcast([128, EC]))
            nc.sync.dma_start(out=bp_sb, in_=bp.ap().to_broadcast([128, D]))
            for eo in range(EC // 128):
                nc.gpsimd.dma_start(
                    out=wp_sb[:, eo, :],
                    in_=wp.ap()[eo * 128:(eo + 1) * 128, :],
                )
            # ones half of augmented v (columns 0:64 of each va tile):
            # generated on-chip — a DMA broadcast would burn ~2MB of SBUF
            # write bandwidth in the critical startup window
            for t in range(N_KT):
                nc.gpsimd.memset(va[:, t, :, 0:64], 1.0)

            # ---------------- P1: qkv projection chunk ----------------
            # Each chunk is emitted in 4 pieces interspersed between the
            # previous query block's p-blocks, so PE fills ACT-paced
            # slivers with exactly the groups the NEXT block needs first:
            # piece 0 = [q0,k0] (unblocks block (g+1, 0) scores), piece 1
            # = v (unblocks its attn@v), pieces 2/3 = remaining pairs.
            chunk_xts = {0: xts0}

            def chunk_dma(sc):
                xts = p1x.tile([128, DT, CB], F32R, tag="xts")
                chunk_xts[sc] = xts
                s0 = sc * CB
                for dt in range(DT):
                    nc.sync.dma_start(
                        out=xts[:, dt, :],
                        in_=xT.ap()[dt * 128:(dt + 1) * 128, s0:s0 + CB],
                    )

            def qk_group(sc, et):
                # q,k: out [e-tile, s-chunk] accumulated over d
                s0 = sc * CB
                pqk = psP.tile([128, CB], F32, tag="PP", name="pqk")
                for dt in range(DT):
                    nc.tensor.matmul(
                        pqk,
                        w_sb[:, dt, et * 128:(et + 1) * 128],
                        chunk_xts[sc][:, dt, :],
                        start=(dt == 0),
                        stop=(dt == DT - 1),
                    )
                dst = qT if et < NP else kT
                slab = et if et < NP else et - NP
                nc.vector.tensor_scalar(
                    out=dst[:, slab, s0:s0 + CB],
                    in0=pqk,
                    scalar1=bqk_sb[:, et:et + 1],
                    scalar2=scale if et < NP else 1.0,
                    op0=mybir.AluOpType.add,
                    op1=mybir.AluOpType.mult,
                )

            def v_group(sc, st):
                # v: natural layout [s-tile, e] accumulated over d
                pv = psP.tile([128, EC], F32, tag="PP", name="pv")
                for dt in range(DT):
                    nc.tensor.matmul(
                        pv,
                        chunk_xts[sc][:, dt, st * 128:(st + 1) * 128],
                        w_sb[:, dt, 2 * EC:3 * EC],
                        start=(dt == 0),
                        stop=(dt == DT - 1),
                    )
                nc.vector.tensor_tensor(
                    out=va[:, sc * (CB // 128) + st, :, 64:128],
                    in0=pv.rearrange("p (h e) -> p h e", e=64),
                    in1=bv_sb.rearrange("p (h e) -> p h e", e=64),
                    op=mybir.AluOpType.add,
                )

            def emit_p1_piece(sc, piece):
                if piece == 0:
                    qk_group(sc, 0)
                    qk_group(sc, NP)
                elif piece == 1:
                    for st in range(CB // 128):
                        v_group(sc, st)
                elif piece == 2:
                    qk_group(sc, 1)
                    qk_group(sc, NP + 1)
                    qk_group(sc, 2)
                    qk_group(sc, NP + 2)
                else:
                    qk_group(sc, 3)
                    qk_group(sc, NP + 3)

            # ---------------- P3: output projection group ----------------
            ysb_tiles = {}

            def emit_p3_group(st, dh):
                py = psP.tile([128, QB], F32, tag="PP", name="py")
                for eo in range(EC // 128):
                    nc.tensor.matmul(
                        py,
                        aoT[:, eo, st * 128:(st + 1) * 128],
                        wp_sb[:, eo, dh * QB:(dh + 1) * QB],
                        start=(eo == 0),
                        stop=(eo == EC // 128 - 1),
                    )
                ysb = ysb_tiles[st % 4]
                nc.vector.tensor_tensor(
                    out=ysb[:, dh * QB:(dh + 1) * QB],
                    in0=py,
                    in1=bp_sb[:, dh * QB:(dh + 1) * QB],
                    op=mybir.AluOpType.add,
                )
                if dh == D // QB - 1:
                    nc.gpsimd.dma_start(
                        out=y.ap()[st * 128:(st + 1) * 128, :],
                        in_=ysb,
                    )

            # ---------------- P2: attention block ----------------
            tri_b = bass.AP(
                tensor=tri_sb.tensor,
                offset=tri_sb.offset,
                ap=[tri_sb.ap[0], [0, 2], tri_sb.ap[1]],
            )
            LOOK = 2  # score/exp tiles emitted ahead of attn@v

            def emit_p2_block(g, p):
                q0 = g * QB
                n_t = 4 * (g + 1) if causal else N_KT
                OA = psO.tile([128, QB], F32, tag="OA")
                OB = psO.tile([128, QB], F32, tag="OB")

                def emit_score_exp(t):
                    j = t - 4 * g if causal else -1
                    qlo = 128 * j if j >= 0 else 0
                    SAB = psS.tile([128, 2, QB], F32, tag="SAB")
                    k0 = t * KT
                    nc.tensor.matmul(
                        SAB[:, 0, qlo:],
                        kT[0:64, p, k0:k0 + KT],
                        qT[0:64, p, q0 + qlo:q0 + QB],
                        start=True, stop=True,
                    )
                    nc.tensor.matmul(
                        SAB[:, 1, qlo:],
                        kT[64:128, p, k0:k0 + KT],
                        qT[64:128, p, q0 + qlo:q0 + QB],
                        start=True, stop=True,
                    )
                    eAB = p2e.tile([128, 2, QB], BF16, tag="eAB")
                    nc.scalar.activation(
                        eAB[:, :, qlo:], SAB[:, :, qlo:],
                        mybir.ActivationFunctionType.Exp,
                    )
                    if j >= 0:
                        nc.vector.tensor_tensor(
                            out=eAB[:, :, qlo:qlo + 128],
                            in0=eAB[:, :, qlo:qlo + 128],
                            in1=tri_b,
                            op=mybir.AluOpType.mult,
                        )
                    return qlo, eAB

                def emit_av(t, qlo, eAB):
                    nc.tensor.matmul(
                        OA[:, qlo:],
                        va[:, t, 2 * p, :],
                        eAB[:, 0, qlo:],
                        start=(t == 0), stop=(t == n_t - 1),
                    )
                    nc.tensor.matmul(
                        OB[:, qlo:],
                        va[:, t, 2 * p + 1, :],
                        eAB[:, 1, qlo:],
                        start=(t == 0), stop=(t == n_t - 1),
                    )

                pending = []
                for t in range(n_t):
                    pending.append((t, *emit_score_exp(t)))
                    if len(pending) > LOOK:
                        emit_av(*pending.pop(0))
                for item in pending:
                    emit_av(*item)

                rcpA = p2r.tile([64, QB], F32, tag="rcpA")
                rcpB = p2r.tile([64, QB], F32, tag="rcpB")
                nc.vector.reciprocal_approx_fast(out=rcpA, in_=OA[0:64, :])
                nc.vector.reciprocal_approx_fast(out=rcpB, in_=OB[0:64, :])
                nc.vector.tensor_tensor(
                    out=aoT[0:64, p, q0:q0 + QB],
                    in0=OA[64:128, :],
                    in1=rcpA,
                    op=mybir.AluOpType.mult,
                )
                nc.vector.tensor_tensor(
                    out=aoT[64:128, p, q0:q0 + QB],
                    in0=OB[64:128, :],
                    in1=rcpB,
                    op=mybir.AluOpType.mult,
                )

            # ---------------- interleaved emission ----------------
            # Chunk g+1's pieces are emitted one per p-block of query
            # block g: priority sits just below that p-block's attention
            # work, so the PE drains them in ACT-paced slivers and the
            # data block g+1 needs first is ready exactly when ACT runs
            # out of block-g exps.
            # P1 chunks and P3 groups are emitted at BACKGROUND priority
            # (large negative offset): their emission position still
            # defines the dependence direction, but the PE only runs them
            # when no P2 score/attn work is ready. Without this the baked
            # queue order puts filler matmuls ahead of the next block's
            # scores and ACT starves ~4us at every block boundary. P3
            # outranks the chunks so it drains in slivers instead of
            # piling into a serial tail after the last exp.
            BG_CHUNK = -1 << 20
            BG_P3 = -1 << 19
            p3_queue = []  # (st, dh) groups awaiting emission
            for g in range(N_QB):
                if g == 0:
                    with tc.high_priority(offset=BG_CHUNK):
                        for piece in range(4):
                            emit_p1_piece(0, piece)
                for p in range(NP):
                    emit_p2_block(g, p)
                    with tc.high_priority(offset=BG_CHUNK):
                        if g + 1 < N_CH:
                            if p == 0:
                                chunk_dma(g + 1)
                            emit_p1_piece(g + 1, p)
                    with tc.high_priority(offset=BG_P3):
                        # deferred output-projection groups from the
                        # previous query block; drain deeper late in the
                        # kernel so no serial pile remains after the exps
                        for _ in range(2 + g):
                            if p3_queue:
                                emit_p3_group(*p3_queue.pop(0))
                # queue this block's output projection
                for st in range(4 * g, 4 * (g + 1)):
                    ysb_tiles[st % 4] = p3y.tile(
                        [128, D], F32, tag="ysb", name="ysb"
                    )
                    for dh in range(D // QB):
                        p3_queue.append((st, dh))
            while p3_queue:
                emit_p3_group(*p3_queue.pop(0))

    nc.compile()
    return nc


def _get_nc(causal: bool):
    if causal not in _nc_cache:
        _nc_cache[causal] = _build(causal)
    return _nc_cache[causal]


def _numpy_fallback(x, mask, w_attn, b_attn, w_proj, b_proj):
    x64 = x.astype(np.float64)
    qkv = x64 @ w_attn.astype(np.float64) + b_attn.astype(np.float64)
    q, k, v = np.split(qkv, 3, axis=-1)
    sp = lambda t: t.reshape(B, S, H, HD).transpose(0, 2, 1, 3)
    q, k, v = sp(q), sp(k), sp(v)
    scores = np.einsum("bhqd,bhkd->bhqk", q, k) / math.sqrt(HD)
    m = np.broadcast_to(np.asarray(mask, bool), scores.shape)
    scores = np.where(m, scores, -np.inf)
    scores -= scores.max(axis=-1, keepdims=True)
    e = np.exp(scores)
    attn = e / e.sum(axis=-1, keepdims=True)
    out = np.einsum("bhqk,bhkd->bhqd", attn, v)
    out = out.transpose(0, 2, 1, 3).reshape(B, S, D)
    return (out @ w_proj.astype(np.float64) + b_proj.astype(np.float64)).astype(
        np.float32
    )


def kernel(x, mask, w_attn, b_attn, w_proj, b_proj) -> np.ndarray:
    from concourse.bass_utils import run_bass_kernel_spmd

    x = np.asarray(x, dtype=np.float32)
    w_attn = np.asarray(w_attn, dtype=np.float32)
    b_attn = np.asarray(b_attn, dtype=np.float32)
    w_proj = np.asarray(w_proj, dtype=np.float32)
    b_proj = np.asarray(b_proj, dtype=np.float32)

    m2 = np.asarray(mask, dtype=bool).reshape(S, S)
    if np.array_equal(m2, np.tril(np.ones((S, S), dtype=bool))):
        causal = True
    elif m2.all():
        causal = False
    else:
        return _numpy_fallback(x, mask, w_attn, b_attn, w_proj, b_proj)

    nc = _get_nc(causal)

    tri_np = np.triu(np.ones((128, 128), dtype=np.float32)).astype(
        ml_dtypes.bfloat16
    )

    in_maps = []
    for c in range(8):
        b, hg = divmod(c, 2)
        e0 = hg * EC
        q_sl = slice(e0, e0 + EC)
        k_sl = slice(D + e0, D + e0 + EC)
        v_sl = slice(2 * D + e0, 2 * D + e0 + EC)
        wq = w_attn[:, q_sl]
        wk = w_attn[:, k_sl]
        wv = w_attn[:, v_sl]
        # device evac computes (q_psum + bias) * scale for q tiles, so the
        # raw biases are passed
        bqk_np = np.concatenate([b_attn[q_sl], b_attn[k_sl]]).reshape(
            2 * EC // 128, 128).T
        in_maps.append({
            "xT": _round_fp32r(x[b].T),
            "wqkv": _round_fp32r(np.concatenate([wq, wk, wv], axis=1)),
            "bqk": np.ascontiguousarray(bqk_np, dtype=np.float32),
            "bv": b_attn[v_sl].reshape(1, EC).copy(),
            "wp": w_proj[q_sl, :].astype(ml_dtypes.bfloat16),
            "bp": (b_proj if hg == 0 else np.zeros_like(b_proj)).reshape(1, D).copy(),
            "tri": tri_np,
        })

    trace = os.environ.get("KERNEL_TRACE") == "1"
    res = run_bass_kernel_spmd(nc, in_maps, core_ids=list(range(8)), trace=trace)
    global last_exec_time_ns
    if res.exec_time_ns is not None:
        last_exec_time_ns = res.exec_time_ns
    parts = [res.results[c]["y"] for c in range(8)]
    out = np.empty((B, S, D), dtype=np.float32)
    for b in range(B):
        out[b] = parts[2 * b] + parts[2 * b + 1]
    return out
